# revision 3
# baseline (speedup 1.0000x reference)
"""Trainium2 Bass kernel for nn_CLFBlock (linear -> LIF scan -> linear -> T-mean -> log_softmax).

Self-contained: hardcodes shapes T=32, B=512, D=1024, C=1000 and data-parallel
sharding of the batch dim across 8 NeuronCores.

Math notes:
  h = x @ W1.T + b1                      (computed in bf16 on the PE, fp32 accum)
  LIF (tau=2, v_th=1, hard reset to 0):
     v' = v + (h - v)/2 = 0.5*v + 0.5*h
     s  = (v' >= 1);  v = v' * (v' < 1)
  y = mean_t(s_t @ W2.T + b2) = (sum_t s_t) @ W2.T / 32 + b2
  out = log_softmax(y, axis=1)
The spike sum is accumulated on the tensor engine: msum_psum += I @ m_t where
m_t = (v' < 1), and sum_t s_t = 32 - msum.
"""

import numpy as np
from contextlib import ExitStack

import concourse.bass as bass
import concourse.tile as tile
from concourse import bacc, mybir
from concourse.bass_utils import run_bass_kernel_spmd

N_CORES = 8
T, B, D, C = 32, 512, 1024, 1000
BC = B // N_CORES          # 64 rows per core
TB = T * BC                # 2048 matmul rows per core
FP32 = mybir.dt.float32
BF16 = mybir.dt.bfloat16
AF = mybir.ActivationFunctionType
OP = mybir.AluOpType


def build_program():
    nc = bacc.Bacc("TRN2", target_bir_lowering=False, debug=False, num_devices=N_CORES)

    x_d = nc.dram_tensor("x", [TB, D], FP32, kind="ExternalInput").ap()
    w1_d = nc.dram_tensor("W1", [D, D], FP32, kind="ExternalInput").ap()
    b1_d = nc.dram_tensor("b1", [D], FP32, kind="ExternalInput").ap()
    w2_d = nc.dram_tensor("W2", [C, D], FP32, kind="ExternalInput").ap()
    b2_d = nc.dram_tensor("b2", [C], FP32, kind="ExternalInput").ap()
    y_d = nc.dram_tensor("y", [BC, C], FP32, kind="ExternalOutput").ap()

    with tile.TileContext(nc) as tc, ExitStack() as ctx:
        persist = ctx.enter_context(tc.tile_pool(name="persist", bufs=1))
        stage = ctx.enter_context(tc.tile_pool(name="stage", bufs=3))
        mpool = ctx.enter_context(tc.tile_pool(name="mpool", bufs=T))
        small = ctx.enter_context(tc.tile_pool(name="small", bufs=1))
        ps_h = ctx.enter_context(tc.tile_pool(name="ps_h", bufs=4, space="PSUM"))
        ps_ms = ctx.enter_context(tc.tile_pool(name="ps_ms", bufs=1, space="PSUM"))
        ps_y = ctx.enter_context(tc.tile_pool(name="ps_y", bufs=2, space="PSUM"))

        # ---- constants / biases ----
        io = small.tile([128, 128], mybir.dt.int32)
        nc.gpsimd.iota(io[:], pattern=[[1, 128]], base=0, channel_multiplier=-1)
        ident = small.tile([128, 128], BF16)
        nc.vector.tensor_scalar(ident[:], io[:], 0, None, op0=OP.is_equal)

        b1_sb = small.tile([128, 8], FP32)
        nc.sync.dma_start(b1_sb[:], b1_d.rearrange("(j p) -> p j", p=128))
        b1h = small.tile([128, 8], FP32)
        nc.vector.tensor_scalar_mul(b1h[:], b1_sb[:], 0.5)

        b2_sb = small.tile([1, C], FP32)
        nc.sync.dma_start(b2_sb[:], b2_d.rearrange("(a c) -> a c", a=1))
        b2_32 = small.tile([1, C], BF16)
        nc.scalar.activation(b2_32[:], b2_sb[:], AF.Copy, scale=float(T))
        ones = small.tile([1, BC], BF16)
        nc.vector.memset(ones[:], 1.0)

        # ---- W1: load+cast then transpose -> W1T[dp, dj*1024 + e] ----
        w1t = persist.tile([128, 8 * 1024], BF16)
        w1t3 = w1t[:].rearrange("p (j e) -> p j e", j=8)
        for ci in range(2):
            w1_bf = stage.tile([128, 4 * 1024], BF16, tag="stage")
            nc.gpsimd.dma_start(
                w1_bf[:].rearrange("p (ii d) -> p ii d", ii=4),
                w1_d[ci * 512:(ci + 1) * 512, :].rearrange("(ii p) d -> p ii d", p=128),
            )
            for ii in range(4):
                i = ci * 4 + ii
                nc.sync.dma_start_transpose(
                    w1t3[:, :, i * 128:(i + 1) * 128],
                    w1_bf[:, ii * 1024:(ii + 1) * 1024],
                )

        # ---- W2: load+cast then transpose -> W2T[ep, ej*1024 + c] ----
        w2t = persist.tile([128, 8 * 1024], BF16)
        w2t3 = w2t[:].rearrange("p (j c) -> p j c", j=8)
        for ci in range(2):
            w2_bf = stage.tile([128, 4 * 1024], BF16, tag="stage")
            if ci == 1:
                # rows 512..1000 only: last 128-row tile is partial (104 rows)
                nc.vector.memset(w2_bf[:, 3 * 1024:4 * 1024], 0.0)
                nc.gpsimd.dma_start(
                    w2_bf[:, 0:3 * 1024].rearrange("p (ii d) -> p ii d", ii=3),
                    w2_d[512:896, :].rearrange("(ii p) d -> p ii d", p=128),
                )
                nc.gpsimd.dma_start(
                    w2_bf[0:104, 3 * 1024:4 * 1024],
                    w2_d[896:1000, :],
                )
            else:
                nc.gpsimd.dma_start(
                    w2_bf[:].rearrange("p (ii d) -> p ii d", ii=4),
                    w2_d[0:512, :].rearrange("(ii p) d -> p ii d", p=128),
                )
            for ii in range(4):
                i = ci * 4 + ii
                nc.sync.dma_start_transpose(
                    w2t3[:, :, i * 128:(i + 1) * 128],
                    w2_bf[:, ii * 1024:(ii + 1) * 1024],
                )

        # ---- x: load+cast then transpose -> xT[dp, dj*2048 + tb] ----
        xt = persist.tile([128, 8 * TB], BF16)
        xt3 = xt[:].rearrange("p (j t) -> p j t", j=8)
        for ci in range(4):
            x_bf = stage.tile([128, 4 * 1024], BF16, tag="stage")
            nc.gpsimd.dma_start(
                x_bf[:].rearrange("p (ii d) -> p ii d", ii=4),
                x_d[ci * 512:(ci + 1) * 512, :].rearrange("(ii p) d -> p ii d", p=128),
            )
            for ii in range(4):
                i = ci * 4 + ii
                nc.sync.dma_start_transpose(
                    xt3[:, :, i * 128:(i + 1) * 128],
                    x_bf[:, ii * 1024:(ii + 1) * 1024],
                )

        # ---- matmul1: h[e, tb] = W1 @ x.T, fused 0.5*h + 0.5*b1 into scan layout ----
        # h_sb free index = t*512 + j*64 + b
        h_sb = persist.tile([128, T * 512], BF16)
        h3 = h_sb[:].rearrange("p (t x) -> p t x", x=512)
        for k in range(4):            # tb chunks of 512 (8 timesteps each)
            for j in range(8):        # e tiles
                ps = ps_h.tile([128, 512], FP32, tag="ps_h")
                for di in range(8):   # contraction tiles
                    nc.tensor.matmul(
                        ps[:],
                        w1t[:, di * 1024 + j * 128: di * 1024 + (j + 1) * 128],
                        xt[:, di * TB + k * 512: di * TB + (k + 1) * 512],
                        start=(di == 0), stop=(di == 7),
                    )
                nc.scalar.activation(
                    h3[:, 8 * k:8 * k + 8, j * 64:(j + 1) * 64],
                    ps[:].rearrange("p (t b) -> p t b", t=8),
                    AF.Identity, scale=0.5, bias=b1h[:, j:j + 1],
                )

        # ---- LIF scan over T steps ----
        vv = small.tile([128, 512], BF16)
        w = small.tile([128, 512], BF16)
        nc.vector.memset(vv[:], 0.0)
        msum = ps_ms.tile([128, 512], FP32)
        for t in range(T):
            h_t = h_sb[:, t * 512:(t + 1) * 512]
            nc.vector.scalar_tensor_tensor(w[:], vv[:], 0.5, h_t, op0=OP.mult, op1=OP.add)
            m = mpool.tile([128, 512], BF16, tag="m")
            nc.vector.tensor_scalar(m[:], w[:], 1.0, None, op0=OP.is_lt)
            nc.vector.tensor_mul(vv[:], w[:], m[:])
            nc.tensor.matmul(msum[:], ident[:], m[:], start=(t == 0), stop=(t == T - 1))

        ssum = small.tile([128, 512], BF16)
        nc.scalar.activation(ssum[:], msum[:], AF.Copy, scale=-1.0, bias=float(T))

        # ---- matmul2: y = ssum @ W2.T / T + b2 ----
        y_sb = small.tile([BC, 1024], FP32)
        for half in range(2):
            n = 512 if half == 0 else C - 512
            c0 = half * 512
            psy = ps_y.tile([BC, 512], FP32, tag="ps_y")
            for ej in range(8):
                nc.tensor.matmul(
                    psy[:, 0:n],
                    ssum[:, ej * 64:(ej + 1) * 64],
                    w2t[:, ej * 1024 + c0: ej * 1024 + c0 + n],
                    start=(ej == 0), stop=False,
                )
            nc.tensor.matmul(psy[:, 0:n], ones[:], b2_32[:, c0:c0 + n],
                             start=False, stop=True)
            nc.scalar.activation(y_sb[:, c0:c0 + n], psy[:, 0:n], AF.Copy,
                                 scale=1.0 / T)

        # ---- log_softmax over C ----
        mx = small.tile([BC, 1], FP32)
        nc.vector.reduce_max(mx[:], y_sb[:, 0:C], axis=mybir.AxisListType.X)
        z = small.tile([BC, 1024], FP32)
        nc.vector.tensor_scalar(z[:, 0:C], y_sb[:, 0:C], mx[:], None, op0=OP.subtract)
        ez = small.tile([BC, 1024], FP32)
        nc.scalar.activation(ez[:, 0:C], z[:, 0:C], AF.Exp)
        ssum_e = small.tile([BC, 1], FP32)
        nc.vector.reduce_sum(ssum_e[:], ez[:, 0:C], axis=mybir.AxisListType.X)
        lse = small.tile([BC, 1], FP32)
        nc.scalar.activation(lse[:], ssum_e[:], AF.Ln)
        out_sb = small.tile([BC, C], FP32)
        nc.vector.tensor_scalar(out_sb[:], z[:, 0:C], lse[:], None, op0=OP.subtract)
        nc.sync.dma_start(y_d[:], out_sb[:])

    nc.compile()
    return nc


_CACHE = {}


def kernel(x, W1, b1, W2, b2):
    if "nc" not in _CACHE:
        _CACHE["nc"] = build_program()
    nc = _CACHE["nc"]

    x = np.ascontiguousarray(x, dtype=np.float32)
    in_maps = []
    for i in range(N_CORES):
        xs = np.ascontiguousarray(x[:, i * BC:(i + 1) * BC, :]).reshape(TB, D)
        in_maps.append({
            "x": xs,
            "W1": np.ascontiguousarray(W1, dtype=np.float32),
            "b1": np.ascontiguousarray(b1, dtype=np.float32),
            "W2": np.ascontiguousarray(W2, dtype=np.float32),
            "b2": np.ascontiguousarray(b2, dtype=np.float32),
        })

    res = run_bass_kernel_spmd(nc, in_maps, core_ids=list(range(N_CORES)),
                               **_CACHE.get("run_kwargs", {}))
    _CACHE["last_results"] = res
    out = np.concatenate([res.results[i]["y"] for i in range(N_CORES)], axis=0)
    return out


# revision 8
# speedup vs baseline: 1.2039x; 1.2039x over previous
"""Trainium2 Bass kernel for nn_CLFBlock (linear -> LIF scan -> linear -> T-mean -> log_softmax).

Self-contained: hardcodes shapes T=32, B=512, D=1024, C=1000 and data-parallel
sharding of the batch dim across 8 NeuronCores.

Math notes:
  h = x @ W1.T + b1                      (computed in bf16 on the PE, fp32 accum)
  LIF (tau=2, v_th=1, hard reset to 0):
     v' = 0.5*v + 0.5*h
     s  = (v' >= 1);  v = v' * (v' < 1)
  Scan state is kept pre-halved:  hh = 0.5*h + 0.5*b1,  vh = 0.5*v, so per step
     w  = vh + hh            (tensor_tensor add, 2x DVE mode)
     mh = (w < 1) * 0.5      (one fused tensor_scalar, 4x DVE mode)
     vh = w * mh             (tensor_tensor mult, 2x DVE mode)
  Spike sum accumulates on the tensor engine: msum_psum += I @ mh_t, and
  sum_t s_t = T - 2*msum.
  y = mean_t(s_t @ W2.T + b2) = (sum_t s_t) @ W2.T / T + b2
  out = log_softmax(y, axis=1)
"""

import numpy as np
from contextlib import ExitStack

import concourse.bass as bass
import concourse.tile as tile
from concourse import bacc, mybir
from concourse.bass_utils import run_bass_kernel_spmd

N_CORES = 8
T, B, D, C = 32, 512, 1024, 1000
BC = B // N_CORES          # 64 rows per core
TB = T * BC                # 2048 matmul rows per core
FP32 = mybir.dt.float32
BF16 = mybir.dt.bfloat16
AF = mybir.ActivationFunctionType
OP = mybir.AluOpType


def build_program():
    nc = bacc.Bacc("TRN2", target_bir_lowering=False, debug=False, num_devices=N_CORES)

    x_d = nc.dram_tensor("x", [TB, D], FP32, kind="ExternalInput").ap()
    w1_d = nc.dram_tensor("W1", [D, D], FP32, kind="ExternalInput").ap()
    b1_d = nc.dram_tensor("b1", [D], FP32, kind="ExternalInput").ap()
    w2_d = nc.dram_tensor("W2", [C, D], FP32, kind="ExternalInput").ap()
    b2_d = nc.dram_tensor("b2", [C], FP32, kind="ExternalInput").ap()
    y_d = nc.dram_tensor("y", [BC, C], FP32, kind="ExternalOutput").ap()

    with tile.TileContext(nc) as tc, ExitStack() as ctx:
        persist = ctx.enter_context(tc.tile_pool(name="persist", bufs=1))
        stage = ctx.enter_context(tc.tile_pool(name="stage", bufs=3))
        mpool = ctx.enter_context(tc.tile_pool(name="mpool", bufs=T))
        small = ctx.enter_context(tc.tile_pool(name="small", bufs=1))
        ps_h = ctx.enter_context(tc.tile_pool(name="ps_h", bufs=4, space="PSUM"))
        ps_ms = ctx.enter_context(tc.tile_pool(name="ps_ms", bufs=1, space="PSUM"))
        ps_y = ctx.enter_context(tc.tile_pool(name="ps_y", bufs=2, space="PSUM"))

        # xbar transposes all on the sync HWDGE queue (concurrent transposes on
        # two queues corrupt data; scalar-queue dispatch also stalls ACT compute)
        tq = [nc.sync, nc.sync]

        # ---- constants / biases needed early ----
        io = small.tile([128, 128], mybir.dt.int32)
        nc.gpsimd.iota(io[:], pattern=[[1, 128]], base=0, channel_multiplier=-1)
        ident = small.tile([128, 128], BF16)
        nc.vector.tensor_scalar(ident[:], io[:], 0, None, op0=OP.is_equal)

        b1_sb = small.tile([128, 8], FP32)
        nc.sync.dma_start(b1_sb[:], b1_d.rearrange("(j p) -> p j", p=128))
        b1h = small.tile([128, 8], FP32)
        nc.vector.tensor_scalar_mul(b1h[:], b1_sb[:], 0.5)

        # ---- W1: load+cast then transpose -> W1T[dp, dj*1024 + e] ----
        w1t = persist.tile([128, 8 * 1024], BF16)
        w1t3 = w1t[:].rearrange("p (j e) -> p j e", j=8)

        def load_w1_chunk(ci):
            w1_bf = stage.tile([128, 4 * 1024], BF16, tag="stage")
            nc.gpsimd.dma_start(
                w1_bf[:].rearrange("p (ii d) -> p ii d", ii=4),
                w1_d[ci * 512:(ci + 1) * 512, :].rearrange("(ii p) d -> p ii d", p=128),
            )
            for ii in range(4):
                i = ci * 4 + ii
                tq[i % 2].dma_start_transpose(
                    w1t3[:, :, i * 128:(i + 1) * 128],
                    w1_bf[:, ii * 1024:(ii + 1) * 1024],
                )

        # ---- x: load+cast then transpose -> xT[dp, dj*2048 + tb] ----
        xt = persist.tile([128, 8 * TB], BF16)
        xt3 = xt[:].rearrange("p (j t) -> p j t", j=8)

        def load_x_chunk(ci):
            x_bf = stage.tile([128, 4 * 1024], BF16, tag="stage")
            nc.gpsimd.dma_start(
                x_bf[:].rearrange("p (ii d) -> p ii d", ii=4),
                x_d[ci * 512:(ci + 1) * 512, :].rearrange("(ii p) d -> p ii d", p=128),
            )
            for ii in range(4):
                i = ci * 4 + ii
                tq[i % 2].dma_start_transpose(
                    xt3[:, :, i * 128:(i + 1) * 128],
                    x_bf[:, ii * 1024:(ii + 1) * 1024],
                )

        load_w1_chunk(0)
        load_x_chunk(0)
        load_w1_chunk(1)
        load_x_chunk(1)

        # ---- matmul1: h[e, tb] = W1 @ x.T, fused 0.5*h + 0.5*b1 into scan layout ----
        # h_sb free index = t*512 + j*64 + b
        h_sb = persist.tile([128, T * 512], BF16)
        h3 = h_sb[:].rearrange("p (t x) -> p t x", x=512)

        def mm1_group(k):
            for j in range(8):
                ps = ps_h.tile([128, 512], FP32, tag="ps_h", name=f"psh_{k}_{j}")
                for di in range(8):
                    nc.tensor.matmul(
                        ps[:],
                        w1t[:, di * 1024 + j * 128: di * 1024 + (j + 1) * 128],
                        xt[:, di * TB + k * 512: di * TB + (k + 1) * 512],
                        start=(di == 0), stop=(di == 7),
                    )
                nc.scalar.activation(
                    h3[:, 8 * k:8 * k + 8, j * 64:(j + 1) * 64],
                    ps[:].rearrange("p (t b) -> p t b", t=8),
                    AF.Identity, scale=0.5, bias=b1h[:, j:j + 1],
                )

        mm1_group(0)
        load_x_chunk(2)
        mm1_group(1)
        load_x_chunk(3)
        mm1_group(2)
        mm1_group(3)

        # ---- LIF scan over T steps ----
        vh = small.tile([128, 512], BF16)   # 0.5 * v
        w = small.tile([128, 512], BF16)
        nc.vector.memset(vh[:], 0.0)
        msum = ps_ms.tile([128, 512], FP32)
        for t in range(T):
            h_t = h_sb[:, t * 512:(t + 1) * 512]
            nc.vector.tensor_add(w[:], vh[:], h_t)
            m = mpool.tile([128, 512], BF16, tag="m")
            nc.vector.tensor_scalar(m[:], w[:], 1.0, 0.5, op0=OP.is_lt, op1=OP.mult)
            nc.vector.tensor_mul(vh[:], w[:], m[:])
            nc.tensor.matmul(msum[:], ident[:], m[:], start=(t == 0), stop=(t == T - 1))

        # sum_t s_t = T - 2 * msum
        ssum = small.tile([128, 512], BF16)
        nc.scalar.activation(ssum[:], msum[:], AF.Copy, scale=-2.0, bias=float(T))

        # ---- W2 (only needed now): load+cast then transpose -> W2T[ep, ej*1024 + c] ----
        w2t = persist.tile([128, 8 * 1024], BF16)
        w2t3 = w2t[:].rearrange("p (j c) -> p j c", j=8)
        for ci in range(2):
            w2_bf = stage.tile([128, 4 * 1024], BF16, tag="stage")
            if ci == 1:
                # rows 512..1000 only: last 128-row tile is partial (104 rows)
                nc.vector.memset(w2_bf[:, 3 * 1024:4 * 1024], 0.0)
                nc.gpsimd.dma_start(
                    w2_bf[:, 0:3 * 1024].rearrange("p (ii d) -> p ii d", ii=3),
                    w2_d[512:896, :].rearrange("(ii p) d -> p ii d", p=128),
                )
                nc.gpsimd.dma_start(
                    w2_bf[0:104, 3 * 1024:4 * 1024],
                    w2_d[896:1000, :],
                )
            else:
                nc.gpsimd.dma_start(
                    w2_bf[:].rearrange("p (ii d) -> p ii d", ii=4),
                    w2_d[0:512, :].rearrange("(ii p) d -> p ii d", p=128),
                )
            for ii in range(4):
                i = ci * 4 + ii
                tq[i % 2].dma_start_transpose(
                    w2t3[:, :, i * 128:(i + 1) * 128],
                    w2_bf[:, ii * 1024:(ii + 1) * 1024],
                )

        b2_sb = small.tile([1, C], FP32)
        nc.sync.dma_start(b2_sb[:], b2_d.rearrange("(a c) -> a c", a=1))
        b2_32 = small.tile([1, C], BF16)
        nc.scalar.activation(b2_32[:], b2_sb[:], AF.Copy, scale=float(T))
        ones = small.tile([1, BC], BF16)
        nc.vector.memset(ones[:], 1.0)

        # ---- matmul2: y = ssum @ W2.T / T + b2 ----
        y_sb = small.tile([BC, 1024], FP32)
        for half in range(2):
            n = 512 if half == 0 else C - 512
            c0 = half * 512
            psy = ps_y.tile([BC, 512], FP32, tag="ps_y")
            for ej in range(8):
                nc.tensor.matmul(
                    psy[:, 0:n],
                    ssum[:, ej * 64:(ej + 1) * 64],
                    w2t[:, ej * 1024 + c0: ej * 1024 + c0 + n],
                    start=(ej == 0), stop=False,
                )
            nc.tensor.matmul(psy[:, 0:n], ones[:], b2_32[:, c0:c0 + n],
                             start=False, stop=True)
            nc.scalar.activation(y_sb[:, c0:c0 + n], psy[:, 0:n], AF.Copy,
                                 scale=1.0 / T)

        # ---- log_softmax over C ----
        mx = small.tile([BC, 1], FP32)
        nc.vector.reduce_max(mx[:], y_sb[:, 0:C], axis=mybir.AxisListType.X)
        z = small.tile([BC, 1024], FP32)
        nc.vector.tensor_scalar(z[:, 0:C], y_sb[:, 0:C], mx[:], None, op0=OP.subtract)
        ez = small.tile([BC, 1024], FP32)
        nc.scalar.activation(ez[:, 0:C], z[:, 0:C], AF.Exp)
        ssum_e = small.tile([BC, 1], FP32)
        nc.vector.reduce_sum(ssum_e[:], ez[:, 0:C], axis=mybir.AxisListType.X)
        lse = small.tile([BC, 1], FP32)
        nc.scalar.activation(lse[:], ssum_e[:], AF.Ln)
        out_sb = small.tile([BC, C], FP32)
        nc.vector.tensor_scalar(out_sb[:], z[:, 0:C], lse[:], None, op0=OP.subtract)
        nc.sync.dma_start(y_d[:], out_sb[:])

    nc.compile()
    return nc


_CACHE = {}


def kernel(x, W1, b1, W2, b2):
    if "nc" not in _CACHE:
        _CACHE["nc"] = build_program()
    nc = _CACHE["nc"]

    x = np.ascontiguousarray(x, dtype=np.float32)
    in_maps = []
    for i in range(N_CORES):
        xs = np.ascontiguousarray(x[:, i * BC:(i + 1) * BC, :]).reshape(TB, D)
        in_maps.append({
            "x": xs,
            "W1": np.ascontiguousarray(W1, dtype=np.float32),
            "b1": np.ascontiguousarray(b1, dtype=np.float32),
            "W2": np.ascontiguousarray(W2, dtype=np.float32),
            "b2": np.ascontiguousarray(b2, dtype=np.float32),
        })

    res = run_bass_kernel_spmd(nc, in_maps, core_ids=list(range(N_CORES)),
                               **_CACHE.get("run_kwargs", {}))
    _CACHE["last_results"] = res
    out = np.concatenate([res.results[i]["y"] for i in range(N_CORES)], axis=0)
    return out
